# revision 15
# baseline (speedup 1.0000x reference)
"""Trainium2 Bass kernel for the ExemplarHead classification problem (v8).

Math: per (task, way), with R the 5x1024 class reps (support+noise),
H = I - (1/5)11^T, G = H R R^T H, the SVD head reduces exactly to
    C = W R,  W = I - lam * (lam I + G)^{-1} H
    logits[q,(w,s)] = (2 q.C - ||q||^2 - ||C||^2) / d
(lam I + G) inverse via one scaled Newton step. All 20 (task,way) blocks
per core are one masked block-diagonal 100x100 problem.

v9 changes vs v8 (27.9us measured):
 - z matmul operand order swapped: hy1^T qq = al lam H Y1 qq = al lam
   H Y2 exactly (no commute approximation) -> recovers precision.
 - cB moved to the sync ring ahead of sn (H lands ~9.2us); qt alone on
   the SWDGE ring; G's diag matmuls moved after the rct chunks so the
   scheduler no longer hoists the derived-consts ahead of the R adds.
 - both chunk-reduce trees moved to the idle GpSimd engine.
 - 12 warmup matmuls (9 spanned 3.06us, just under the 3.4us HAM
   window - the whole v8 kernel ran at 1.2GHz).

Sharding: data-parallel over the 32 tasks -> 4 tasks per NeuronCore x 8.
"""

import numpy as np
import ml_dtypes

import concourse.bass as bass
import concourse.mybir as mybir
import concourse.tile as tile
from concourse import bacc
from concourse.bass_utils import run_bass_kernel_spmd

F32 = mybir.dt.float32
BF16 = mybir.dt.bfloat16
FP16 = mybir.dt.float16
FP8 = mybir.dt.float8e4
AF = mybir.ActivationFunctionType
ALU = mybir.AluOpType

LAM = 100000.0
GMAX_BOUND = 40000.0            # safe bound on ||G|| (observed max ~2.2e4)
ALPHA = 2.0 / (2.0 * LAM + GMAX_BOUND)

N_CORES = 8
T_FULL, NQ, D = 32, 75, 1024
NW, NS = 5, 5
TPC = T_FULL // N_CORES          # tasks per core = 4
NR = TPC * NW * NS               # R rows per core = 100
NCH = D // 128                   # 8 contraction chunks
NJ = NW * NS                     # 25 (way,shot) pairs per task
NQT = TPC * NQ                   # 300 (task,query) columns per core
CB_COLS = 400                    # bf16 const tile columns
N_WARM = 11                      # PE warmup matmuls
WN = 384                         # warmup matmul free size


def _host_consts():
    """cB bf16 [100,400]: H (block-diag), alpha*lam*H, I, alpha*blockmask."""
    H5 = np.eye(NS) - np.ones((NS, NS)) / NS
    H_bd = np.kron(np.eye(TPC * NW), H5).astype(np.float32)       # [100,100]
    blockmask = np.kron(np.eye(TPC * NW), np.ones((NS, NS))).astype(np.float32)
    eye = np.eye(NR, dtype=np.float32)
    cB = np.zeros((NR, CB_COLS), dtype=np.float32)
    cB[:, 0:NR] = H_bd
    cB[:, NR:2 * NR] = ALPHA * LAM * H_bd
    cB[:, 2 * NR:3 * NR] = eye
    cB[:, 3 * NR:4 * NR] = ALPHA * blockmask
    return cB.astype(ml_dtypes.bfloat16)


def build_nc():
    nc = bacc.Bacc("TRN2")

    qt_d = nc.declare_dram_parameter("qt", [128, NCH * NQT], FP8,
                                     isOutput=False)
    rzc_d = nc.declare_dram_parameter("rzc", [NR, 2 * D + CB_COLS], BF16,
                                      isOutput=False)
    out_d = nc.declare_dram_parameter("out", [NR, NQT], FP16, isOutput=True)

    with tile.TileContext(nc) as tc:
        with (
            tc.tile_pool(name="consts", bufs=1) as consts,
            tc.tile_pool(name="sb", bufs=1) as sb,
            tc.tile_pool(name="pipe", bufs=3, space="PSUM") as pipe,
            tc.tile_pool(name="gp", bufs=1, space="PSUM") as gp,
            tc.tile_pool(name="cnp", bufs=1, space="PSUM") as cnp,
            tc.tile_pool(name="qnp", bufs=1, space="PSUM") as qnp,
            tc.tile_pool(name="qcp", bufs=1, space="PSUM") as qcp,
            tc.tile_pool(name="wp", bufs=1, space="PSUM") as wp,
        ):
            # ---- input DMAs: one combined R-path DMA + qt SWDGE ----
            rzc = sb.tile([NR, 2 * D + CB_COLS], BF16)
            qtb = sb.tile([128, NCH * NQT], BF16)
            nc.sync.dma_start(out=rzc, in_=rzc_d[:])
            nc.gpsimd.dma_start(out=qtb, in_=qt_d[:])      # SWDGE fp8->bf16
            sn_sb = rzc[:, 0:D]
            nz_sb = rzc[:, D:2 * D]
            cB = rzc[:, 2 * D:2 * D + CB_COLS]
            c_Hb = cB[:, 0:NR]
            c_alHb = cB[:, NR:2 * NR]
            c_I = cB[:, 2 * NR:3 * NR]
            c_amask = cB[:, 3 * NR:4 * NR]

            # ---- memset consts (DVE, early) ----
            wsrc = sb.tile([128, WN], BF16)
            nc.vector.memset(wsrc, 0.0)
            onescol = sb.tile([128, 1], FP16)
            nc.vector.memset(onescol, 1.0)
            neghcol = sb.tile([128, 1], FP16)
            nc.vector.memset(neghcol, -0.5)
            ones300 = sb.tile([1, NQT], FP16)
            nc.vector.memset(ones300, 1.0)

            # ---- PE warmup: release the HAM clock gate before real work ----
            w_ps = wp.tile([128, WN], F32, space="PSUM")
            for i in range(N_WARM):
                nc.tensor.matmul(w_ps, lhsT=wsrc[:, 0:128], rhs=wsrc,
                                 start=True, stop=True)

            # ---- R = support + noise on DVE (bf16, halves) ----
            HD = D // 2
            rb = sb.tile([NR, D], BF16)
            for h in range(2):
                sl = slice(h * HD, (h + 1) * HD)
                nc.vector.tensor_add(rb[:, sl], sn_sb[:, sl], nz_sb[:, sl])

            # derived const matrices (need cB)
            d316 = sb.tile([NR, NR], BF16)
            nc.vector.tensor_scalar(d316, c_I, 316.0, None, ALU.mult)
            d12 = sb.tile([NR, NR], BF16)
            nc.vector.tensor_scalar(d12, c_I, 12.0, None, ALU.mult)
            twoI = sb.tile([NR, NR], BF16)
            nc.vector.tensor_scalar(twoI, c_I, 2.0, None, ALU.mult)
            twoalH = sb.tile([NR, NR], BF16)
            nc.vector.tensor_scalar(twoalH, c_alHb, 2.0, None, ALU.mult)
            pair = sb.tile([NR, 2 * NR], BF16)
            nc.vector.tensor_copy(pair[:, NR:2 * NR], c_alHb)

            # ---- sq = qt.^2 fp16 (Scalar ACT, 2 halves) for ||q||^2 ----
            sq = sb.tile([128, NCH * NQT], FP16)
            HQ = NCH * NQT // 2
            for h in range(2):
                sl = slice(h * HQ, (h + 1) * HQ)
                nc.scalar.activation(sq[:, sl], qtb[:, sl], AF.Square)

            # ---- RcT = (H R)^T by chunks (bf16) ----
            rctb = sb.tile([128, NCH * NR], BF16)
            for p in range(2):
                rct_ps = pipe.tile([128, 4 * NR], F32, space="PSUM", tag="pp")
                for kk in range(4):
                    k = 4 * p + kk
                    nc.tensor.matmul(rct_ps[:, kk * NR:(kk + 1) * NR],
                                     lhsT=rb[:, k * 128:(k + 1) * 128],
                                     rhs=c_Hb, start=True, stop=True)
                nc.vector.tensor_copy(rctb[:, p * 4 * NR:(p + 1) * 4 * NR],
                                      rct_ps)

            # ---- G + lam*I in one psum (diag matmuls are exact) ----
            g_ps = gp.tile([NR, NR], F32, space="PSUM")
            for k in range(NCH):
                rct_k = rctb[:, k * NR:(k + 1) * NR]
                nc.tensor.matmul(g_ps, lhsT=rct_k, rhs=rct_k,
                                 start=(k == 0), stop=False)
            nc.tensor.matmul(g_ps, lhsT=d316, rhs=d316, start=False,
                             stop=False)
            nc.tensor.matmul(g_ps, lhsT=d12, rhs=d12, start=False, stop=True)

            # ---- Ka (masked), short Newton, W^T ----
            ka_b = sb.tile([NR, NR], BF16)
            nc.vector.tensor_mul(ka_b, g_ps, c_amask)      # Ka = amask*(G+lamI)
            y1_b = pair[:, 0:NR]
            nc.vector.tensor_sub(y1_b, twoI, ka_b)         # Y1 = 2I - Ka
            phk_ps = pipe.tile([NR, 2 * NR], F32, space="PSUM", tag="pp")
            nc.tensor.matmul(phk_ps, lhsT=ka_b, rhs=pair, start=True,
                             stop=True)                    # [Ka Y1 | Ka alH]
            nc.tensor.matmul(w_ps[0:NR, :], lhsT=ka_b, rhs=wsrc[0:NR, :],
                             start=True, stop=True)        # HAM filler
            qq_b = sb.tile([NR, NR], BF16)
            nc.vector.tensor_sub(qq_b, twoI, phk_ps[:, 0:NR])
            hy1_b = sb.tile([NR, NR], BF16)
            nc.vector.tensor_sub(hy1_b, twoalH, phk_ps[:, NR:2 * NR])
            z_ps = pipe.tile([NR, NR], F32, space="PSUM", tag="pp")
            nc.tensor.matmul(z_ps, lhsT=hy1_b, rhs=qq_b, start=True,
                             stop=True)                    # al lam H Y2
            nc.tensor.matmul(w_ps[0:NR, :], lhsT=qq_b, rhs=wsrc[0:NR, :],
                             start=True, stop=True)        # HAM filler
            wt_b = sb.tile([NR, NR], BF16)
            nc.vector.tensor_sub(wt_b, c_I, z_ps)          # W^T

            # ---- C^T chunks (bf16) + fp16 squares for ||C||^2 ----
            ctb = sb.tile([128, NCH * NR], BF16)
            csqb = sb.tile([128, NCH * NR], FP16)
            for p in range(2):
                ct_ps = pipe.tile([128, 4 * NR], F32, space="PSUM", tag="pp")
                for kk in range(4):
                    k = 4 * p + kk
                    nc.tensor.matmul(ct_ps[:, kk * NR:(kk + 1) * NR],
                                     lhsT=rb[:, k * 128:(k + 1) * 128],
                                     rhs=wt_b, start=True, stop=True)
                sl = slice(p * 4 * NR, (p + 1) * 4 * NR)
                if p == 0:
                    nc.tensor.matmul(w_ps[0:NR, :], lhsT=wt_b,
                                     rhs=wsrc[0:NR, :], start=True, stop=True)
                nc.vector.tensor_copy(ctb[:, sl], ct_ps)
                nc.scalar.activation(csqb[:, sl], ct_ps, AF.Square)

            # ---- DVE tree-reduce chunk sums (fp16) ----
            # sqr[p,c] = sum_k sq[p, k*300+c]  -> one matmul for ||q||^2
            sqr4 = sb.tile([128, 4 * NQT], FP16)
            for j in range(4):
                nc.gpsimd.tensor_add(sqr4[:, j * NQT:(j + 1) * NQT],
                                     sq[:, 2 * j * NQT:(2 * j + 1) * NQT],
                                     sq[:, (2 * j + 1) * NQT:(2 * j + 2) * NQT])
            sqr2 = sb.tile([128, 2 * NQT], FP16)
            for j in range(2):
                nc.gpsimd.tensor_add(sqr2[:, j * NQT:(j + 1) * NQT],
                                     sqr4[:, 2 * j * NQT:(2 * j + 1) * NQT],
                                     sqr4[:, (2 * j + 1) * NQT:(2 * j + 2) * NQT])
            sqr = sb.tile([128, NQT], FP16)
            nc.gpsimd.tensor_add(sqr, sqr2[:, 0:NQT], sqr2[:, NQT:2 * NQT])
            # csum[p,j] = sum_k csq[p, k*100+j] -> one matmul for ||C||^2
            csum4 = sb.tile([128, 4 * NR], FP16)
            for j in range(4):
                eng = nc.vector if j % 2 == 0 else nc.gpsimd
                eng.tensor_add(csum4[:, j * NR:(j + 1) * NR],
                               csqb[:, 2 * j * NR:(2 * j + 1) * NR],
                               csqb[:, (2 * j + 1) * NR:(2 * j + 2) * NR])
            csum2 = sb.tile([128, 2 * NR], FP16)
            for j in range(2):
                eng = nc.vector if j % 2 == 0 else nc.gpsimd
                eng.tensor_add(csum2[:, j * NR:(j + 1) * NR],
                               csum4[:, 2 * j * NR:(2 * j + 1) * NR],
                               csum4[:, (2 * j + 1) * NR:(2 * j + 2) * NR])
            csum = sb.tile([128, NR], FP16)
            nc.vector.tensor_add(csum, csum2[:, 0:NR], csum2[:, NR:2 * NR])

            # ---- single-matmul ||q||^2 row ----
            qn_ps = qnp.tile([1, NQT], F32, space="PSUM")
            nc.tensor.matmul(qn_ps, lhsT=onescol, rhs=sqr, start=True,
                             stop=True)
            qnh = sb.tile([1, NQT], FP16)
            nc.scalar.activation(qnh, qn_ps, AF.Copy, scale=-0.5)

            # ---- QC transposed: psum[(t,j),(t,q)] = C q^T + qn fold ----
            ones100 = ones300[0:1, 0:NR]
            qc_ps = qcp.tile([NR, NQT], F32, space="PSUM")
            cn_ps = cnp.tile([NR, 1], F32, space="PSUM")
            cncol = sb.tile([NR, 1], F32)
            for k in range(NCH):
                nc.tensor.matmul(qc_ps, lhsT=ctb[:, k * NR:(k + 1) * NR],
                                 rhs=qtb[:, k * NQT:(k + 1) * NQT],
                                 start=(k == 0), stop=(k == NCH - 1))
                if k == 4:
                    # ||C||^2 as a COLUMN: folds into the epilogue
                    nc.tensor.matmul(cn_ps, lhsT=csum, rhs=neghcol,
                                     start=True, stop=True)
                if k == 6:
                    nc.tensor.matmul(qc_ps, lhsT=ones100, rhs=qnh,
                                     start=False, stop=False)
            nc.vector.tensor_copy(cncol, cn_ps)

            # ---- epilogue: (psum + cn) * 2/D, DMA out ----
            out_sb = sb.tile([NR, NQT], FP16)
            nc.vector.tensor_scalar(out_sb, qc_ps, cncol, 2.0 / D,
                                    ALU.add, ALU.mult)
            nc.sync.dma_start(out=out_d[:], in_=out_sb)

    nc.finalize()
    return nc


_NC_CACHE = None


def _get_nc():
    global _NC_CACHE
    if _NC_CACHE is None:
        _NC_CACHE = build_nc()
    return _NC_CACHE


def make_in_maps(query, support, noise):
    query = np.asarray(query, dtype=np.float32)
    support = np.asarray(support, dtype=np.float32)
    noise = np.asarray(noise, dtype=np.float32)
    cB = _host_consts().astype(np.float32)
    in_maps = []
    for c in range(N_CORES):
        ts = slice(c * TPC, (c + 1) * TPC)
        qc = query[ts]                                   # (4, 75, 1024)
        # qt[p, k*300 + t*75 + q] = q[t, q, 128k+p]
        qt = np.ascontiguousarray(
            qc.transpose(2, 0, 1).reshape(NCH, 128, NQT)
              .transpose(1, 0, 2).reshape(128, NCH * NQT)
        ).astype(ml_dtypes.float8_e4m3)
        sn = support[ts].reshape(NR, D)
        nz = noise[:, ts].transpose(1, 0, 2, 3).reshape(NR, D)
        rzc = np.concatenate(
            [sn, nz, cB.astype(np.float32)], axis=1
        ).astype(ml_dtypes.bfloat16)
        in_maps.append({"qt": qt, "rzc": np.ascontiguousarray(rzc)})
    return in_maps


def kernel(query, support, noise, support_labels=None, n_way=None, n_shot=None,
           **_unused):
    nc = _get_nc()
    in_maps = make_in_maps(query, support, noise)
    res = run_bass_kernel_spmd(nc, in_maps, list(range(N_CORES)))
    # out is [(t,j), (t',q)] = [100, 300]; take diagonal task blocks,
    # then (4, 25, 75) -> (4, 75, 25)
    outs = []
    for r in res.results:
        o = np.asarray(r["out"]).astype(np.float32).reshape(TPC, NJ, TPC, NQ)
        blk = o[np.arange(TPC), :, np.arange(TPC), :]   # (4, 25, 75)
        outs.append(blk.transpose(0, 2, 1))
    full = np.concatenate(outs, axis=0)            # (32, 75, 25)
    return full.reshape(T_FULL, NQ, NW, NS).astype(np.float32)


# revision 16
# speedup vs baseline: 1.0669x; 1.0669x over previous
"""Trainium2 Bass kernel for the ExemplarHead classification problem (v8).

Math: per (task, way), with R the 5x1024 class reps (support+noise),
H = I - (1/5)11^T, G = H R R^T H, the SVD head reduces exactly to
    C = W R,  W = I - lam * (lam I + G)^{-1} H
    logits[q,(w,s)] = (2 q.C - ||q||^2 - ||C||^2) / d
(lam I + G) inverse via one scaled Newton step. All 20 (task,way) blocks
per core are one masked block-diagonal 100x100 problem.

v9 changes vs v8 (27.9us measured):
 - z matmul operand order swapped: hy1^T qq = al lam H Y1 qq = al lam
   H Y2 exactly (no commute approximation) -> recovers precision.
 - cB moved to the sync ring ahead of sn (H lands ~9.2us); qt alone on
   the SWDGE ring; G's diag matmuls moved after the rct chunks so the
   scheduler no longer hoists the derived-consts ahead of the R adds.
 - both chunk-reduce trees moved to the idle GpSimd engine.
 - 12 warmup matmuls (9 spanned 3.06us, just under the 3.4us HAM
   window - the whole v8 kernel ran at 1.2GHz).

Sharding: data-parallel over the 32 tasks -> 4 tasks per NeuronCore x 8.
"""

import numpy as np
import ml_dtypes

import concourse.bass as bass
import concourse.mybir as mybir
import concourse.tile as tile
from concourse import bacc
from concourse.bass_utils import run_bass_kernel_spmd

F32 = mybir.dt.float32
BF16 = mybir.dt.bfloat16
FP16 = mybir.dt.float16
FP8 = mybir.dt.float8e4
AF = mybir.ActivationFunctionType
ALU = mybir.AluOpType

LAM = 100000.0
GMAX_BOUND = 40000.0            # safe bound on ||G|| (observed max ~2.2e4)
ALPHA = 2.0 / (2.0 * LAM + GMAX_BOUND)

N_CORES = 8
T_FULL, NQ, D = 32, 75, 1024
NW, NS = 5, 5
TPC = T_FULL // N_CORES          # tasks per core = 4
NR = TPC * NW * NS               # R rows per core = 100
NCH = D // 128                   # 8 contraction chunks
NJ = NW * NS                     # 25 (way,shot) pairs per task
NQT = TPC * NQ                   # 300 (task,query) columns per core
CB_COLS = 400                    # bf16 const tile columns
N_WARM = 15                      # PE warmup matmuls
WN = 384                         # warmup matmul free size


def _host_consts():
    """cB bf16 [100,400]: H (block-diag), alpha*lam*H, I, alpha*blockmask."""
    H5 = np.eye(NS) - np.ones((NS, NS)) / NS
    H_bd = np.kron(np.eye(TPC * NW), H5).astype(np.float32)       # [100,100]
    blockmask = np.kron(np.eye(TPC * NW), np.ones((NS, NS))).astype(np.float32)
    eye = np.eye(NR, dtype=np.float32)
    cB = np.zeros((NR, CB_COLS), dtype=np.float32)
    cB[:, 0:NR] = H_bd
    cB[:, NR:2 * NR] = ALPHA * LAM * H_bd
    cB[:, 2 * NR:3 * NR] = eye
    cB[:, 3 * NR:4 * NR] = ALPHA * blockmask
    return cB.astype(ml_dtypes.bfloat16)


def build_nc():
    nc = bacc.Bacc("TRN2")

    qt_d = nc.declare_dram_parameter("qt", [128, NCH * NQT], FP8,
                                     isOutput=False)
    sn_d = nc.declare_dram_parameter("sn", [NR, D], BF16, isOutput=False)
    nz_d = nc.declare_dram_parameter("nz", [NR, D], BF16, isOutput=False)
    cB_d = nc.declare_dram_parameter("cB", [NR, CB_COLS], BF16, isOutput=False)
    out_d = nc.declare_dram_parameter("out", [NR, NQT], FP16, isOutput=True)

    with tile.TileContext(nc) as tc:
        with (
            tc.tile_pool(name="consts", bufs=1) as consts,
            tc.tile_pool(name="sb", bufs=1) as sb,
            tc.tile_pool(name="pipe", bufs=3, space="PSUM") as pipe,
            tc.tile_pool(name="gp", bufs=1, space="PSUM") as gp,
            tc.tile_pool(name="cnp", bufs=1, space="PSUM") as cnp,
            tc.tile_pool(name="qnp", bufs=1, space="PSUM") as qnp,
            tc.tile_pool(name="qcp", bufs=1, space="PSUM") as qcp,
            tc.tile_pool(name="wp", bufs=1, space="PSUM") as wp,
        ):
            # ---- input DMAs: 3 parallel paths ----
            cB = consts.tile([NR, CB_COLS], BF16)
            sn_sb = sb.tile([NR, D], BF16)
            nz_sb = sb.tile([NR, D], BF16)
            qtb = sb.tile([128, NCH * NQT], BF16)
            nc.sync.dma_start(out=sn_sb, in_=sn_d[:])
            nc.scalar.dma_start(out=nz_sb, in_=nz_d[:])
            nc.gpsimd.dma_start(out=cB, in_=cB_d[:])
            nc.gpsimd.dma_start(out=qtb, in_=qt_d[:])      # SWDGE fp8->bf16
            c_Hb = cB[:, 0:NR]
            c_alHb = cB[:, NR:2 * NR]
            c_I = cB[:, 2 * NR:3 * NR]
            c_amask = cB[:, 3 * NR:4 * NR]

            # ---- memset consts (DVE, early) ----
            wsrc = sb.tile([128, WN], BF16)
            nc.vector.memset(wsrc, 0.0)
            onescol = sb.tile([128, 1], FP16)
            nc.vector.memset(onescol, 1.0)
            neghcol = sb.tile([128, 1], FP16)
            nc.vector.memset(neghcol, -0.5)
            ones300 = sb.tile([1, NQT], FP16)
            nc.vector.memset(ones300, 1.0)

            # ---- PE warmup: release the HAM clock gate before real work ----
            w_ps = wp.tile([128, WN], F32, space="PSUM")
            for i in range(N_WARM):
                nc.tensor.matmul(w_ps, lhsT=wsrc[:, 0:128], rhs=wsrc,
                                 start=True, stop=True)

            # ---- R = support + noise on DVE (bf16, halves) ----
            HD = D // 2
            rb = sb.tile([NR, D], BF16)
            for h in range(2):
                sl = slice(h * HD, (h + 1) * HD)
                nc.vector.tensor_add(rb[:, sl], sn_sb[:, sl], nz_sb[:, sl])

            # derived const matrices (need cB)
            d316 = sb.tile([NR, NR], BF16)
            nc.vector.tensor_scalar(d316, c_I, 316.0, None, ALU.mult)
            d12 = sb.tile([NR, NR], BF16)
            nc.vector.tensor_scalar(d12, c_I, 12.0, None, ALU.mult)
            twoI = sb.tile([NR, NR], BF16)
            nc.vector.tensor_scalar(twoI, c_I, 2.0, None, ALU.mult)
            twoalH = sb.tile([NR, NR], BF16)
            nc.vector.tensor_scalar(twoalH, c_alHb, 2.0, None, ALU.mult)
            pair = sb.tile([NR, 2 * NR], BF16)
            nc.vector.tensor_copy(pair[:, NR:2 * NR], c_alHb)

            # ---- sq = qt.^2 fp16 (Scalar ACT, 2 halves) for ||q||^2 ----
            sq = sb.tile([128, NCH * NQT], FP16)
            HQ = NCH * NQT // 2
            for h in range(2):
                sl = slice(h * HQ, (h + 1) * HQ)
                nc.scalar.activation(sq[:, sl], qtb[:, sl], AF.Square)

            # ---- RcT = (H R)^T by chunks (bf16) ----
            rctb = sb.tile([128, NCH * NR], BF16)
            for p in range(2):
                rct_ps = pipe.tile([128, 4 * NR], F32, space="PSUM", tag="pp")
                for kk in range(4):
                    k = 4 * p + kk
                    nc.tensor.matmul(rct_ps[:, kk * NR:(kk + 1) * NR],
                                     lhsT=rb[:, k * 128:(k + 1) * 128],
                                     rhs=c_Hb, start=True, stop=True)
                nc.vector.tensor_copy(rctb[:, p * 4 * NR:(p + 1) * 4 * NR],
                                      rct_ps)

            # ---- G + lam*I in one psum (diag matmuls are exact) ----
            g_ps = gp.tile([NR, NR], F32, space="PSUM")
            for k in range(NCH):
                rct_k = rctb[:, k * NR:(k + 1) * NR]
                nc.tensor.matmul(g_ps, lhsT=rct_k, rhs=rct_k,
                                 start=(k == 0), stop=False)
            nc.tensor.matmul(g_ps, lhsT=d316, rhs=d316, start=False,
                             stop=False)
            nc.tensor.matmul(g_ps, lhsT=d12, rhs=d12, start=False, stop=True)

            # ---- Ka (masked), short Newton, W^T ----
            ka_b = sb.tile([NR, NR], BF16)
            nc.vector.tensor_mul(ka_b, g_ps, c_amask)      # Ka = amask*(G+lamI)
            y1_b = pair[:, 0:NR]
            nc.vector.tensor_sub(y1_b, twoI, ka_b)         # Y1 = 2I - Ka
            phk_ps = pipe.tile([NR, 2 * NR], F32, space="PSUM", tag="pp")
            nc.tensor.matmul(phk_ps, lhsT=ka_b, rhs=pair, start=True,
                             stop=True)                    # [Ka Y1 | Ka alH]
            qq_b = sb.tile([NR, NR], BF16)
            nc.vector.tensor_sub(qq_b, twoI, phk_ps[:, 0:NR])
            hy1_b = sb.tile([NR, NR], BF16)
            nc.vector.tensor_sub(hy1_b, twoalH, phk_ps[:, NR:2 * NR])
            z_ps = pipe.tile([NR, NR], F32, space="PSUM", tag="pp")
            nc.tensor.matmul(z_ps, lhsT=hy1_b, rhs=qq_b, start=True,
                             stop=True)                    # al lam H Y2
            wt_b = sb.tile([NR, NR], BF16)
            nc.vector.tensor_sub(wt_b, c_I, z_ps)          # W^T

            # ---- C^T chunks (bf16) + fp16 squares for ||C||^2 ----
            ctb = sb.tile([128, NCH * NR], BF16)
            csqb = sb.tile([128, NCH * NR], FP16)
            for p in range(2):
                ct_ps = pipe.tile([128, 4 * NR], F32, space="PSUM", tag="pp")
                for kk in range(4):
                    k = 4 * p + kk
                    nc.tensor.matmul(ct_ps[:, kk * NR:(kk + 1) * NR],
                                     lhsT=rb[:, k * 128:(k + 1) * 128],
                                     rhs=wt_b, start=True, stop=True)
                sl = slice(p * 4 * NR, (p + 1) * 4 * NR)
                nc.vector.tensor_copy(ctb[:, sl], ct_ps)
                nc.scalar.activation(csqb[:, sl], ct_ps, AF.Square)

            # ---- DVE tree-reduce chunk sums (fp16) ----
            # sqr[p,c] = sum_k sq[p, k*300+c]  -> one matmul for ||q||^2
            sqr4 = sb.tile([128, 4 * NQT], FP16)
            for j in range(4):
                nc.gpsimd.tensor_add(sqr4[:, j * NQT:(j + 1) * NQT],
                                     sq[:, 2 * j * NQT:(2 * j + 1) * NQT],
                                     sq[:, (2 * j + 1) * NQT:(2 * j + 2) * NQT])
            sqr2 = sb.tile([128, 2 * NQT], FP16)
            for j in range(2):
                nc.gpsimd.tensor_add(sqr2[:, j * NQT:(j + 1) * NQT],
                                     sqr4[:, 2 * j * NQT:(2 * j + 1) * NQT],
                                     sqr4[:, (2 * j + 1) * NQT:(2 * j + 2) * NQT])
            sqr = sb.tile([128, NQT], FP16)
            nc.gpsimd.tensor_add(sqr, sqr2[:, 0:NQT], sqr2[:, NQT:2 * NQT])
            # csum[p,j] = sum_k csq[p, k*100+j] -> one matmul for ||C||^2
            csum4 = sb.tile([128, 4 * NR], FP16)
            for j in range(4):
                eng = nc.vector if j % 2 == 0 else nc.gpsimd
                eng.tensor_add(csum4[:, j * NR:(j + 1) * NR],
                               csqb[:, 2 * j * NR:(2 * j + 1) * NR],
                               csqb[:, (2 * j + 1) * NR:(2 * j + 2) * NR])
            csum2 = sb.tile([128, 2 * NR], FP16)
            for j in range(2):
                eng = nc.vector if j % 2 == 0 else nc.gpsimd
                eng.tensor_add(csum2[:, j * NR:(j + 1) * NR],
                               csum4[:, 2 * j * NR:(2 * j + 1) * NR],
                               csum4[:, (2 * j + 1) * NR:(2 * j + 2) * NR])
            csum = sb.tile([128, NR], FP16)
            nc.vector.tensor_add(csum, csum2[:, 0:NR], csum2[:, NR:2 * NR])

            # ---- single-matmul ||q||^2 row ----
            qn_ps = qnp.tile([1, NQT], F32, space="PSUM")
            nc.tensor.matmul(qn_ps, lhsT=onescol, rhs=sqr, start=True,
                             stop=True)
            qnh = sb.tile([1, NQT], FP16)
            nc.scalar.activation(qnh, qn_ps, AF.Copy, scale=-0.5)

            # ---- QC transposed: psum[(t,j),(t,q)] = C q^T + qn fold ----
            ones100 = ones300[0:1, 0:NR]
            qc_ps = qcp.tile([NR, NQT], F32, space="PSUM")
            cn_ps = cnp.tile([NR, 1], F32, space="PSUM")
            cncol = sb.tile([NR, 1], F32)
            for k in range(NCH):
                nc.tensor.matmul(qc_ps, lhsT=ctb[:, k * NR:(k + 1) * NR],
                                 rhs=qtb[:, k * NQT:(k + 1) * NQT],
                                 start=(k == 0), stop=(k == NCH - 1))
                if k == 4:
                    # ||C||^2 as a COLUMN: folds into the epilogue
                    nc.tensor.matmul(cn_ps, lhsT=csum, rhs=neghcol,
                                     start=True, stop=True)
                if k == 6:
                    nc.tensor.matmul(qc_ps, lhsT=ones100, rhs=qnh,
                                     start=False, stop=False)
            nc.vector.tensor_copy(cncol, cn_ps)

            # ---- epilogue: (psum + cn) * 2/D, DMA out ----
            out_sb = sb.tile([NR, NQT], FP16)
            nc.vector.tensor_scalar(out_sb, qc_ps, cncol, 2.0 / D,
                                    ALU.add, ALU.mult)
            nc.sync.dma_start(out=out_d[:], in_=out_sb)

    nc.finalize()
    return nc


_NC_CACHE = None


def _get_nc():
    global _NC_CACHE
    if _NC_CACHE is None:
        _NC_CACHE = build_nc()
    return _NC_CACHE


def make_in_maps(query, support, noise):
    query = np.asarray(query, dtype=np.float32)
    support = np.asarray(support, dtype=np.float32)
    noise = np.asarray(noise, dtype=np.float32)
    cB = _host_consts().astype(np.float32)
    in_maps = []
    for c in range(N_CORES):
        ts = slice(c * TPC, (c + 1) * TPC)
        qc = query[ts]                                   # (4, 75, 1024)
        # qt[p, k*300 + t*75 + q] = q[t, q, 128k+p]
        qt = np.ascontiguousarray(
            qc.transpose(2, 0, 1).reshape(NCH, 128, NQT)
              .transpose(1, 0, 2).reshape(128, NCH * NQT)
        ).astype(ml_dtypes.float8_e4m3)
        in_maps.append({
            "qt": qt,
            "sn": np.ascontiguousarray(support[ts]).reshape(NR, D)
                  .astype(ml_dtypes.bfloat16),
            "nz": np.ascontiguousarray(
                noise[:, ts].transpose(1, 0, 2, 3)).reshape(NR, D)
                  .astype(ml_dtypes.bfloat16),
            "cB": cB.astype(ml_dtypes.bfloat16),
        })
    return in_maps


def kernel(query, support, noise, support_labels=None, n_way=None, n_shot=None,
           **_unused):
    nc = _get_nc()
    in_maps = make_in_maps(query, support, noise)
    res = run_bass_kernel_spmd(nc, in_maps, list(range(N_CORES)))
    # out is [(t,j), (t',q)] = [100, 300]; take diagonal task blocks,
    # then (4, 25, 75) -> (4, 75, 25)
    outs = []
    for r in res.results:
        o = np.asarray(r["out"]).astype(np.float32).reshape(TPC, NJ, TPC, NQ)
        blk = o[np.arange(TPC), :, np.arange(TPC), :]   # (4, 25, 75)
        outs.append(blk.transpose(0, 2, 1))
    full = np.concatenate(outs, axis=0)            # (32, 75, 25)
    return full.reshape(T_FULL, NQ, NW, NS).astype(np.float32)
